# revision 16
# baseline (speedup 1.0000x reference)
"""Trainium2 Bass kernel for nn_Decoder_5480378270296 (fp8 DoubleRow version).

Two-layer GRU decoder with argmax-feedback embedding lookup, data-parallel
over 8 NeuronCores: the flattened msl*bs=8192 batch is split into 8 shards
of 1024 rows; all parameters are replicated.

All recurrent matmuls run in fp8e4m3 with MatmulPerfMode.DoubleRow (two
128-row contraction blocks per instruction -> 157 TF/s, 2x the fp32r rate
the previous version used). Weights are pre-scaled by 64 on the host; the
1/64 descale is folded into the activation-function scale operand. The
hidden state itself is stored in fp8 (scale 1) so matmul inputs need no
extra conversion ops; numpy simulation of this exact quantization scheme
gives rel err ~4.4e-3 vs the fp32 reference (gate is 2e-2).

The log-softmax -ln(sum(exp)) correction is batched every 7 steps so the
ACT engine's sigmoid/tanh table never swaps inside the recurrence.
"""
import sys
import numpy as np
import ml_dtypes

for _p in ("/root/.axon_site/_ro/trn_rl_repo", "/opt/trn_rl_repo"):
    if _p not in sys.path:
        sys.path.append(_p)

import concourse.bass as bass  # noqa: E402
import concourse.bacc as bacc  # noqa: E402
import concourse.mybir as mybir  # noqa: E402
from concourse import tile  # noqa: E402
from concourse.bass_utils import run_bass_kernel_spmd  # noqa: E402

F32 = mybir.dt.float32
BF16 = mybir.dt.bfloat16
F8 = mybir.dt.float8e4
E4M3 = ml_dtypes.float8_e4m3
AF = mybir.ActivationFunctionType
ALU = mybir.AluOpType
AX = mybir.AxisListType
PM = mybir.MatmulPerfMode

MSL, BS, ENC = 64, 128, 1024
HID, EMB, ATOM = 512, 50, 64
MAX_STEPS = 50
SOS = 1
NCORES = 8
B = MSL * BS // NCORES  # 1024 rows per core
NB = 512                # moving columns per matmul
NH = B // NB            # 2 column blocks
KC = HID // 128         # 4 hidden chunks
KP = KC // 2            # 2 DoubleRow k-pairs
KPE = ENC // 256        # 4 encoder k-pairs
CH = B // 128           # 8 batch chunks of 128
STEPS = MAX_STEPS - 1   # 49 outputs
TBLK = 7                # log-softmax correction batch (49 = 7*7)
SW = 64.0               # fp8 weight scale
INV = 1.0 / SW

_CACHE = {}


def _build(steps=STEPS):
    nc = bacc.Bacc(None, target_bir_lowering=False)

    dp = nc.declare_dram_parameter
    enc8p = dp("enc8", [KPE, 128, 2, B], F8, isOutput=False)
    wh08p = dp("wh08", [KPE, 128, 2, 2 * HID], F8, isOutput=False)
    whh0p = dp("whh0", [KP, 128, 2, 3 * HID], F8, isOutput=False)
    wih1p = dp("wih1", [KP, 128, 2, 3 * HID], F8, isOutput=False)
    whh1p = dp("whh1", [KP, 128, 2, 3 * HID], F8, isOutput=False)
    woutp = dp("wout8", [KP, 128, 2, ATOM], F8, isOutput=False)
    wEp = dp("wE8", [ATOM, 3 * HID], F8, isOutput=False)
    bh0p = dp("bh0", [128, 2 * KC], F32, isOutput=False)
    brz0p = dp("brz0", [128, 2 * KC], F32, isOutput=False)
    brz0s0p = dp("brz0s0", [128, 2 * KC], F32, isOutput=False)
    bihn0p = dp("bihn0", [128, KC], F32, isOutput=False)
    bn0s0p = dp("bn0s0", [128, KC], F32, isOutput=False)
    bhhn0sp = dp("bhhn0s", [128, KC], F32, isOutput=False)
    brz1p = dp("brz1", [128, 2 * KC], F32, isOutput=False)
    bihn1p = dp("bihn1", [128, KC], F32, isOutput=False)
    bhhn1sp = dp("bhhn1s", [128, KC], F32, isOutput=False)
    boutBp = dp("boutB", [128, 4 * ATOM], F32, isOutput=False)
    idnbp = dp("idnb", [128, 128], BF16, isOutput=False)
    outp = dp("out", [steps, B, ATOM], F32, isOutput=True)

    with tile.TileContext(nc) as tc:
        with (
            tc.tile_pool(name="state", bufs=1) as st,
            tc.tile_pool(name="psum", bufs=2, space="PSUM") as ps,
        ):
            # persistent fp8 hidden state, paired along the DoubleRow k dim
            h0p = [st.tile([128, 2, B], F8, tag=f"h0p{p}", name=f"h0p{p}")
                   for p in range(KP)]
            h1p = [st.tile([128, 2, B], F8, tag=f"h1p{p}", name=f"h1p{p}")
                   for p in range(KP)]

            # ---- init: h = tanh((w_h0 @ enc) / SW + b_h0), written as fp8 ----
            with tc.tile_pool(name="init", bufs=1) as ip:
                bh0_t = ip.tile([128, 2 * KC], F32, tag="bh0", name="bh0")
                nc.sync.dma_start(out=bh0_t[:], in_=bh0p[:])
                wh0 = []
                enc = []
                for p in range(KPE):
                    tw = ip.tile([128, 2, 2 * HID], F8, tag=f"wh0_{p}",
                                 name=f"wh0_{p}")
                    nc.sync.dma_start(out=tw[:], in_=wh08p[p])
                    wh0.append(tw)
                    te = ip.tile([128, 2, B], F8, tag=f"enc_{p}",
                                 name=f"enc_{p}")
                    nc.sync.dma_start(out=te[:], in_=enc8p[p])
                    enc.append(te)
                for m in range(2 * KC):
                    ms = slice(m * 128, (m + 1) * 128)
                    hp = (h0p if m < KC else h1p)[(m % KC) // 2]
                    j = m % 2
                    for col in range(NH):
                        cs = slice(col * NB, (col + 1) * NB)
                        pi = ps.tile([128, NB], F32, tag="pa", name="pa")
                        for p in range(KPE):
                            nc.tensor.matmul(
                                pi[:], wh0[p][:, :, ms], enc[p][:, :, cs],
                                start=(p == 0), stop=(p == KPE - 1),
                                perf_mode=PM.DoubleRow,
                            )
                        nc.scalar.activation(
                            hp[:, j, cs], pi[:], AF.Tanh,
                            bias=bh0_t[:, m : m + 1], scale=INV,
                        )

            with (
                tc.tile_pool(name="weights", bufs=1) as wp,
                tc.tile_pool(name="work", bufs=2) as wk,
                tc.tile_pool(name="ybuf", bufs=TBLK) as yb,
            ):
                def load_pairs(name, src, mdim):
                    ts = []
                    for p in range(KP):
                        t = wp.tile([128, 2, mdim], F8, tag=f"{name}{p}",
                                    name=f"{name}{p}")
                        nc.sync.dma_start(out=t[:], in_=src[p])
                        ts.append(t)
                    return ts

                whh0 = load_pairs("whh0", whh0p, 3 * HID)
                wih1 = load_pairs("wih1", wih1p, 3 * HID)
                whh1 = load_pairs("whh1", whh1p, 3 * HID)
                wout = load_pairs("wout", woutp, ATOM)
                wE = wp.tile([ATOM, 3 * HID], F8, tag="wE", name="wE")
                nc.sync.dma_start(out=wE[:], in_=wEp[:])

                def load_f32(name, src, shape, dt=F32):
                    t = wp.tile(shape, dt, tag=name, name=name)
                    nc.sync.dma_start(out=t[:], in_=src[:])
                    return t

                brz0_t = load_f32("brz0", brz0p, [128, 2 * KC])
                brz0s0_t = load_f32("brz0s0", brz0s0p, [128, 2 * KC])
                bihn0_t = load_f32("bihn0", bihn0p, [128, KC])
                bn0s0_t = load_f32("bn0s0", bn0s0p, [128, KC])
                bhhn0s_t = load_f32("bhhn0s", bhhn0sp, [128, KC])
                brz1_t = load_f32("brz1", brz1p, [128, 2 * KC])
                bihn1_t = load_f32("bihn1", bihn1p, [128, KC])
                bhhn1s_t = load_f32("bhhn1s", bhhn1sp, [128, KC])
                boutB = load_f32("boutB", boutBp, [128, 4 * ATOM])
                idnb = load_f32("idnb", idnbp, [128, 128], BF16)

                def gru_layer(whh, hp, ihT, brz_t, bihn_t, bhhns_t):
                    """One GRU layer over the full [hidden, B] state.

                    whh: 2 paired weight tiles; hp: 2 paired fp8 state tiles
                    (updated in place). ihT: None (biases pre-folded), a
                    [ATOM, B] fp8 one-hot (layer 0, via wE), or "h0" to use
                    wih1 @ h0p (layer 1). The ih matmuls are emitted one
                    block late so the PE has independent hh work queued while
                    the ih operand (one-hot / fresh h0) is still being made.
                    """
                    use_wE = ihT is not None and ihT != "h0"
                    use_h0 = ihT == "h0"
                    groups = {}
                    pending = None

                    def emit_ih(col_, k_):
                        cs_ = slice(col_ * NB, (col_ + 1) * NB)
                        pr_, pz_, pgin_, _ = groups[(col_, k_)]
                        if use_wE:
                            for (pt, j) in ((pr_, k_), (pz_, k_ + KC)):
                                ms_ = slice(j * 128, (j + 1) * 128)
                                nc.tensor.matmul(pt[:], wE[:, ms_],
                                                 ihT[:, cs_],
                                                 start=False, stop=True)
                            ms_ = slice((k_ + 2 * KC) * 128,
                                        (k_ + 2 * KC + 1) * 128)
                            nc.tensor.matmul(pgin_[:], wE[:, ms_],
                                             ihT[:, cs_],
                                             start=True, stop=True)
                        else:
                            for (pt, j, st) in (
                                (pr_, k_, False), (pz_, k_ + KC, False),
                                (pgin_, k_ + 2 * KC, True),
                            ):
                                ms_ = slice(j * 128, (j + 1) * 128)
                                for p in range(KP):
                                    nc.tensor.matmul(
                                        pt[:], wih1[p][:, :, ms_],
                                        h0p[p][:, :, cs_],
                                        start=(st and p == 0),
                                        stop=(p == KP - 1),
                                        perf_mode=PM.DoubleRow,
                                    )

                    # matmul phase (hh immediate, ih one block deferred)
                    for col in range(NH):
                        cs = slice(col * NB, (col + 1) * NB)
                        for k in range(KC):
                            pr = ps.tile([128, NB], F32, tag="pa", name="pa")
                            pz = ps.tile([128, NB], F32, tag="pb", name="pb")
                            pghn = ps.tile([128, NB], F32, tag="pd", name="pd")
                            pgin = None
                            if use_wE or use_h0:
                                pgin = ps.tile([128, NB], F32, tag="pc",
                                               name="pc")
                            groups[(col, k)] = (pr, pz, pgin, pghn)
                            hh_stop = not (use_wE or use_h0)
                            for (pt, j, stop_ok) in (
                                (pr, k, hh_stop), (pz, k + KC, hh_stop),
                                (pghn, k + 2 * KC, True),
                            ):
                                ms = slice(j * 128, (j + 1) * 128)
                                for p in range(KP):
                                    nc.tensor.matmul(
                                        pt[:], whh[p][:, :, ms],
                                        hp[p][:, :, cs],
                                        start=(p == 0),
                                        stop=(stop_ok and p == KP - 1),
                                        perf_mode=PM.DoubleRow,
                                    )
                            if use_wE or use_h0:
                                if pending is not None:
                                    emit_ih(*pending)
                                pending = (col, k)
                    if pending is not None:
                        emit_ih(*pending)

                    # elementwise phase: everything per (col, k) at [128,512]
                    # so h chunks complete column-incrementally (layer 1's ih
                    # matmuls for col 0 can start while col 1 is still going)
                    for col in range(NH):
                        cs = slice(col * NB, (col + 1) * NB)
                        for k in range(KC):
                            pr, pz, pgin, pghn = groups[(col, k)]
                            r = wk.tile([128, NB], F32, tag="r", name="r",
                                        bufs=3)
                            nc.scalar.activation(r[:], pr[:], AF.Sigmoid,
                                                 bias=brz_t[:, k : k + 1],
                                                 scale=INV)
                            z = wk.tile([128, NB], F32, tag="z", name="z",
                                        bufs=3)
                            nc.scalar.activation(z[:], pz[:],
                                                 AF.Sigmoid,
                                                 bias=brz_t[:, KC + k : KC + k + 1],
                                                 scale=INV)
                            u = wk.tile([128, NB], F32, tag="u", name="u",
                                        bufs=3)
                            nc.vector.scalar_tensor_tensor(
                                u[:], pghn[:], bhhns_t[:, k : k + 1], r[:],
                                ALU.add, ALU.mult,
                            )
                            if pgin is not None:
                                t3 = wk.tile([128, NB], F32, tag="t3",
                                             name="t3", bufs=3)
                                nc.vector.tensor_tensor(t3[:], u[:], pgin[:],
                                                        ALU.add)
                            else:
                                t3 = u
                            n = wk.tile([128, NB], F32, tag="n", name="n",
                                        bufs=3)
                            nc.scalar.activation(n[:], t3[:], AF.Tanh,
                                                 bias=bihn_t[:, k : k + 1],
                                                 scale=INV)
                            hs = hp[k // 2][:, k % 2, cs]
                            d = wk.tile([128, NB], F32, tag="d", name="d",
                                        bufs=3)
                            nc.gpsimd.tensor_tensor(d[:], hs, n[:],
                                                    ALU.subtract)
                            g = wk.tile([128, NB], F32, tag="g", name="g",
                                        bufs=3)
                            if col == 0:
                                nc.gpsimd.tensor_tensor(g[:], z[:], d[:],
                                                        ALU.mult)
                            else:
                                nc.vector.tensor_tensor(g[:], z[:], d[:],
                                                        ALU.mult)
                            nc.vector.tensor_tensor(hs, n[:], g[:], ALU.add)

                ohT_prev = None
                ybatch = []
                for t in range(steps):
                    if t == 0:
                        gru_layer(whh0, h0p, None, brz0s0_t, bn0s0_t,
                                  bhhn0s_t)
                    else:
                        gru_layer(whh0, h0p, ohT_prev, brz0_t, bihn0_t,
                                  bhhn0s_t)
                    gru_layer(whh1, h1p, "h0", brz1_t, bihn1_t, bhhn1s_t)

                    # logits computed DIRECTLY in [batch, atom] layout:
                    # psum[bc] = h1-slice.T @ w_outT (DoubleRow, h1 pair tile
                    # as the stationary operand). No transposes, no ACT.
                    ytile = yb.tile([128, CH, ATOM], F32, tag="y", name="y")
                    if t < steps - 1:
                        ohT8 = wk.tile([ATOM, B], F8, tag="ohT8",
                                       name="ohT8")
                    for half in range(2):
                        pnp = ps.tile([128, 4, ATOM], F32, tag="pb",
                                      name="pb")
                        for c4 in range(4):
                            c = half * 4 + c4
                            bs = slice(c * 128, (c + 1) * 128)
                            for p in range(KP):
                                nc.tensor.matmul(
                                    pnp[:, c4, :], h1p[p][:, :, bs],
                                    wout[p][:],
                                    start=(p == 0), stop=(p == KP - 1),
                                    perf_mode=PM.DoubleRow,
                                )
                        poh = None
                        if t < steps - 1:
                            poh = ps.tile([ATOM, 4, 128], BF16, tag="pc",
                                          name="pc")
                        for c4 in range(4):
                            c = half * 4 + c4
                            lb = wk.tile([128, ATOM], F32, tag="lb",
                                         name="lb", bufs=8)
                            nc.vector.tensor_tensor(lb[:], pnp[:, c4, :],
                                                    boutB[:, ATOM : 2 * ATOM],
                                                    ALU.add)
                            mneg = wk.tile([128, 1], F32, tag="mx",
                                           name="mx", bufs=8)
                            nc.vector.tensor_reduce(mneg[:], lb[:],
                                                    axis=AX.X, op=ALU.max,
                                                    negate=True)
                            nc.vector.tensor_scalar(
                                ytile[:, c, :], lb[:], mneg[:], INV,
                                ALU.add, ALU.mult)
                            if t < steps - 1:
                                ohb = wk.tile([128, ATOM], BF16, tag="ohb",
                                              name="ohb", bufs=8)
                                nc.vector.tensor_scalar(
                                    ohb[:], ytile[:, c, :], 0.0, None,
                                    ALU.is_ge, ALU.bypass)
                                nc.tensor.transpose(
                                    poh[:, c4, :], ohb[:], idnb[:])
                        if t < steps - 1:
                            nc.scalar.copy(
                                ohT8[:, half * 512 : (half + 1) * 512],
                                poh[:].rearrange("p c f -> p (c f)"))
                    if t < steps - 1:
                        ohT_prev = ohT8

                    ybatch.append((t, ytile))

                    # ---- batched log-softmax correction + output DMA ----
                    if len(ybatch) == TBLK or t == steps - 1:
                        s8s = []
                        for (tt, yt) in ybatch:
                            scr = wk.tile([128, CH * ATOM], F32, tag="scr",
                                          name="scr")
                            nc.scalar.activation(
                                scr[:], yt[:].rearrange("p c a -> p (c a)"),
                                AF.Exp)
                            s8 = wk.tile([128, CH], F32, tag="s8", name="s8",
                                         bufs=TBLK)
                            nc.vector.tensor_reduce(
                                s8[:],
                                scr[:].rearrange("p (c a) -> p c a", a=ATOM),
                                axis=AX.X, op=ALU.add)
                            s8s.append(s8)
                        for (tt, yt), s8 in zip(ybatch, s8s):
                            ln8 = wk.tile([128, CH], F32, tag="ln8",
                                          name="ln8", bufs=TBLK)
                            nc.scalar.activation(ln8[:], s8[:], AF.Ln)
                            for c in range(CH):
                                nc.vector.tensor_scalar_sub(
                                    yt[:, c, :], yt[:, c, :],
                                    ln8[:, c : c + 1])
                            nc.sync.dma_start(
                                out=outp[tt].rearrange("(c p) a -> p c a",
                                                       p=128),
                                in_=yt[:])
                        ybatch = []

    nc.compile()
    return nc


def _prep_maps(inputs, steps=STEPS):
    f = {k: np.ascontiguousarray(np.asarray(v, np.float32))
         for k, v in inputs.items()}
    enc_flat = f["encoder_output"].reshape(MSL * BS, ENC)

    def pairs(wT, np_pairs):
        # wT: [K, M] -> [np_pairs, 128, 2, M] fp8 (scaled by SW)
        K, M = wT.shape
        a = np.asarray(wT * SW, E4M3).reshape(np_pairs, 2, 128, M)
        return np.ascontiguousarray(a.transpose(0, 2, 1, 3))

    wE = f["emb"] @ f["w_ih0"].T  # [ATOM, 3H]
    sos0 = f["w_ih0"] @ f["emb"][SOS]  # [3H]

    common = {
        "wh08": pairs(f["w_h0"].T, KPE),
        "whh0": pairs(f["w_hh0"].T, KP),
        "wih1": pairs(f["w_ih1"].T, KP),
        "whh1": pairs(f["w_hh1"].T, KP),
        "wout8": pairs(f["w_out"].T, KP),
        "wE8": np.ascontiguousarray(np.asarray(wE * SW, E4M3)),
        "bh0": np.ascontiguousarray(f["b_h0"].reshape(2 * KC, 128).T),
        "brz0": np.ascontiguousarray(
            (f["b_ih0"] + f["b_hh0"])[: 2 * HID].reshape(2 * KC, 128).T),
        "brz0s0": np.ascontiguousarray(
            ((f["b_ih0"] + f["b_hh0"])[: 2 * HID]
             + sos0[: 2 * HID]).reshape(2 * KC, 128).T),
        "bihn0": np.ascontiguousarray(
            f["b_ih0"][2 * HID :].reshape(KC, 128).T),
        "bn0s0": np.ascontiguousarray(
            (f["b_ih0"][2 * HID :] + sos0[2 * HID :]).reshape(KC, 128).T),
        "bhhn0s": np.ascontiguousarray(
            (f["b_hh0"][2 * HID :] * SW).reshape(KC, 128).T),
        "brz1": np.ascontiguousarray(
            (f["b_ih1"] + f["b_hh1"])[: 2 * HID].reshape(2 * KC, 128).T),
        "bihn1": np.ascontiguousarray(
            f["b_ih1"][2 * HID :].reshape(KC, 128).T),
        "bhhn1s": np.ascontiguousarray(
            (f["b_hh1"][2 * HID :] * SW).reshape(KC, 128).T),
        "boutB": np.ascontiguousarray(
            np.concatenate([
                np.tile(f["b_out"][None, :], (128, 1)),
                np.tile(f["b_out"][None, :] * SW, (128, 1)),
                np.tile(f["b_out"][None, :], (128, 2)),
            ], axis=1)),
        "idnb": np.eye(128, dtype=np.float32).astype(ml_dtypes.bfloat16),
    }
    in_maps = []
    for c in range(NCORES):
        shard = enc_flat[c * B : (c + 1) * B]  # [B, ENC]
        encT = np.asarray(shard.T, E4M3)  # [ENC, B], scale 1
        m = dict(common)
        m["enc8"] = np.ascontiguousarray(
            encT.reshape(KPE, 2, 128, B).transpose(0, 2, 1, 3))
        in_maps.append(m)
    return in_maps


def kernel(**inputs) -> np.ndarray:
    steps = STEPS
    if "nc" not in _CACHE:
        _CACHE["nc"] = _build(steps)
    nc = _CACHE["nc"]
    in_maps = _prep_maps(inputs, steps)
    res = run_bass_kernel_spmd(nc, in_maps, core_ids=list(range(NCORES)))
    parts = [res.results[c]["out"] for c in range(NCORES)]
    full = np.concatenate(parts, axis=1)  # [steps, 8192, 64]
    return np.ascontiguousarray(
        full.reshape(steps, MSL, BS, ATOM).astype(np.float32))


if __name__ == "__main__":
    import time

    t0 = time.time()
    nc = _build(STEPS)
    print(f"build+compile: {time.time() - t0:.1f}s")
